# revision 42
# baseline (speedup 1.0000x reference)
"""Trainium2 Bass kernel for nn_MultiHeadAttn_80126909874682 (v3, fp8 DoubleRow).

Full MHA layer: QKV projection -> 16-head attention (seq 2048) -> output
projection -> residual -> LayerNorm, over h [2048, 4, 1024] fp32.

Sharding (8 NeuronCores, zero collectives):
  core c -> batch b = c // 2, token-half r = c % 2.
  Each core computes K/V for all 2048 tokens of its batch (all 16 heads)
  and Q / attention / output projection / LayerNorm for its 1024 local
  tokens.  The per-core inputs are permuted so local tokens come first.

v3: every matmul runs as an fp8e4 DoubleRow matmul (two 128-row k-tiles
per instruction, 0.5 PE cycles per output row).  Weights and h are
pre-quantized to fp8e4 on the host with a x16 / x1 scale; score descale
(1/2048) and a -5*ln2 bias are folded into the softmax exp; the final
1/256 descale is folded into the residual add.  Softmax exp is split
between the ACT engine (exact exp, fp8 output) and the DVE engine
(one-instruction Schraudolph exp: scores * A + B rounded to uint8 IS
the fp8e4 bit pattern).  Denominators come from a ones column packed
into the PV weight tiles; LayerNorm statistics and tail run on Pool.

DoubleRow ISA constraint honored throughout: the stationary operand
must be [p, 2, 128] with the two 128-column slots contiguous (kt tiles
are duplicated via SBUF->SBUF DMA to satisfy this); the moving operand
tolerates arbitrary slot strides including 0 (broadcast).
"""

import os
import sys

os.environ.setdefault("JAX_PLATFORMS", "axon")
sys.path.insert(0, "/opt/trn_rl_repo")

import numpy as np
import ml_dtypes

import concourse.bass as bass
import concourse.tile as tile
from concourse import bacc, mybir
from concourse.bass import ts
from concourse.bass_utils import run_bass_kernel_spmd

N_HEAD = 16
D_MODEL = 1024
D_HEAD = 64
SEQ = 2048
BATCH = 4
EPS = 1e-5
N_CORES = 8

LOCAL = SEQ // 2            # tokens owned per core (1024)
N_PAIR = N_HEAD // 2        # head pairs (8)
JT = SEQ // 128             # j tiles (16)
JG = JT // 2                # j tile pairs (8)

F32 = mybir.dt.float32
FP8 = mybir.dt.float8e4
U8 = mybir.dt.uint8
AF = mybir.ActivationFunctionType
ALU = mybir.AluOpType
DR = mybir.MatmulPerfMode.DoubleRow
E4 = ml_dtypes.float8_e4m3

LOG2E = 1.4426950408889634
WS = 16.0                       # weight quantization scale
EXP_SCALE = 1.0 / 4096.0        # score descale (16*16*8 * 2 dup slots)
EXP_BIAS = -5.0 * float(np.log(2.0))
SCHD_A = 8.0 * LOG2E * EXP_SCALE
SCHD_B = 56.0 + 8.0 * LOG2E * EXP_BIAS   # = 16.0
OUT_DESCALE = 1.0 / 256.0

# drain scheduling: pick the engine whose accumulated queue cost is lowest,
# so consecutive exp tiles alternate ACT/DVE and run concurrently.
ACT_EXP_NS, DVE_EXP_NS = 1038.0, 1550.0
ACT_CPY_NS, DVE_CPY_NS = 1038.0, 1192.0


class _Sched:
    """Cost-balancing chooser between ACT ('a') and DVE ('d')."""

    def __init__(self):
        self.a_ns = 0.0
        self.d_ns = 0.0   # DVE-only tail work is accounted as emitted

    def pick(self, a_cost=ACT_EXP_NS, d_cost=DVE_EXP_NS):
        if self.a_ns + a_cost <= self.d_ns + d_cost:
            self.a_ns += a_cost
            return "a"
        self.d_ns += d_cost
        return "d"

    def dve_extra(self, ns):
        self.d_ns += ns

    def act_extra(self, ns):
        self.a_ns += ns


DEBUG_TAPS = False


def build_program():
    nc = bacc.Bacc()

    hbt = nc.declare_dram_parameter("hbt", [128, 16384], FP8, isOutput=False)
    wq = nc.declare_dram_parameter("wq", [128, 8192], FP8, isOutput=False)
    wk = nc.declare_dram_parameter("wk", [128, 8192], FP8, isOutput=False)
    wv = nc.declare_dram_parameter("wv", [128, 8192], FP8, isOutput=False)
    wo = nc.declare_dram_parameter("wo", [128, 8192], FP8, isOutput=False)
    hbres = nc.declare_dram_parameter("hbres", [LOCAL, D_MODEL], F32,
                                      isOutput=False)
    gamma = nc.declare_dram_parameter("gamma", [D_MODEL], F32, isOutput=False)
    beta = nc.declare_dram_parameter("beta", [D_MODEL], F32, isOutput=False)
    out = nc.declare_dram_parameter("out", [LOCAL, D_MODEL], F32, isOutput=True)
    taps = None
    if DEBUG_TAPS:
        taps = {
            "t_kt2": nc.declare_dram_parameter("t_kt2", [128, 4096], FP8, isOutput=True),
            "t_qt": nc.declare_dram_parameter("t_qt", [128, 1024], FP8, isOutput=True),
            "t_e": nc.declare_dram_parameter("t_e", [128, 1024], FP8, isOutput=True),
            "t_vall": nc.declare_dram_parameter("t_vall", [128, 2048], FP8, isOutput=True),
            "t_at": nc.declare_dram_parameter("t_at", [128, 4096], FP8, isOutput=True),
            "t_rb": nc.declare_dram_parameter("t_rb", [64, 512], F32, isOutput=True),
            "t_acc": nc.declare_dram_parameter("t_acc", [16, 512], F32, isOutput=True),
            "t_rec": nc.declare_dram_parameter("t_rec", [1, 512], F32, isOutput=True),
        }

    with tile.TileContext(nc) as tc:
        with (
            tc.tile_pool(name="consts", bufs=1) as consts,
            tc.tile_pool(name="hbt", bufs=1) as hbt_pool,
            tc.tile_pool(name="wqk", bufs=1) as wqk_pool,
            tc.tile_pool(name="wvo", bufs=1) as wvo_pool,
            tc.tile_pool(name="vall", bufs=1) as v_pool,
            tc.tile_pool(name="kt2", bufs=2) as kt2_pool,
            tc.tile_pool(name="kttmp", bufs=2) as kttmp_pool,
            tc.tile_pool(name="qt", bufs=2) as qt_pool,
            tc.tile_pool(name="e", bufs=12) as e_pool,
            tc.tile_pool(name="attnT", bufs=2) as at_pool,
            tc.tile_pool(name="x", bufs=4) as x_pool,
            tc.tile_pool(name="hbr", bufs=4) as hbr_pool,
            tc.tile_pool(name="small", bufs=4) as sm_pool,
            tc.tile_pool(name="ps_s2", bufs=3, space="PSUM") as ps_s2,
            tc.tile_pool(name="ps_acc", bufs=1, space="PSUM") as ps_acc,
            tc.tile_pool(name="ps_pp", bufs=1, space="PSUM") as ps_pp,
        ):
            _emit(nc, tc, hbt, wq, wk, wv, wo, hbres, gamma, beta, out,
                  consts, hbt_pool, wqk_pool, wvo_pool, v_pool, kt2_pool,
                  kttmp_pool, qt_pool, e_pool, at_pool, x_pool, hbr_pool,
                  sm_pool, ps_s2, ps_acc, ps_pp, taps)

    nc.finalize()
    return nc


def _emit(nc, tc, hbt_d, wq_d, wk_d, wv_d, wo_d, hbres_d, gamma_d, beta_d,
          out_d, consts, hbt_pool, wqk_pool, wvo_pool, v_pool, kt2_pool,
          kttmp_pool, qt_pool, e_pool, at_pool, x_pool, hbr_pool, sm_pool,
          ps_s2, ps_acc, ps_pp, taps=None):
    sched = _Sched()

    # ---- constants ----
    gamma_b = consts.tile([128, D_MODEL], F32, name="gamma_b")
    beta_b = consts.tile([128, D_MODEL], F32, name="beta_b")
    ebias = consts.tile([128, 1], F32, name="ebias")
    eps_t = consts.tile([128, 1], F32, name="eps")
    nc.vector.memset(ebias[:], EXP_BIAS)
    nc.vector.memset(eps_t[:], EPS)
    # ---- weight / activation DMAs (wv + hbt first: V-proj gating) ----
    hbt = hbt_pool.tile([128, 16384], FP8, name="hbt")
    wq_sb = wqk_pool.tile([128, 8192], FP8, name="wq")
    wk_sb = wqk_pool.tile([128, 8192], FP8, name="wk")
    wv_sb = wvo_pool.tile([128, 8192], FP8, name="wv")
    wo_sb = wvo_pool.tile([128, 8192], FP8, name="wo")
    for c in range(4):
        eng = nc.sync if c % 2 == 0 else nc.gpsimd
        eng.dma_start(wv_sb[:, ts(c, 2048)], wv_d[:, ts(c, 2048)])
    for half in range(2):
        for k in range(4):
            off = k * 4096 + half * 2048
            eng = nc.sync if k % 2 == 0 else nc.gpsimd
            eng.dma_start(hbt[:, off:off + 2048], hbt_d[:, off:off + 2048])
    def emit_weight_dmas():
        for c in range(4):
            eng = nc.sync if c % 2 == 0 else nc.gpsimd
            eng.dma_start(wk_sb[:, ts(c, 2048)], wk_d[:, ts(c, 2048)])
            eng.dma_start(wq_sb[:, ts(c, 2048)], wq_d[:, ts(c, 2048)])
            eng.dma_start(wo_sb[:, ts(c, 2048)], wo_d[:, ts(c, 2048)])

    # ---- v_all: [128 j, 16 heads * 8 jpairs * 256]; per 128-block:
    #      cols 0:64 = 16*v, col 64 = 1.0 (denominator), 65:128 = 0 ----
    v_all = v_pool.tile([128, N_HEAD * 2048], FP8, name="v_all")
    va = v_all[:].rearrange("p (b c) -> p b c", c=128)     # [128, 256, 128]
    nc.gpsimd.memset(va[:, :, 64:65], 1.0)
    nc.gpsimd.memset(va[:, :, 65:128], 0.0)

    def hbt_pair(k, tb):
        """hbt [p, 2, 128] weights view for dm-chunk pair k, token block tb."""
        base = k * 4096 + tb * 256
        return hbt[:, base:base + 256].rearrange("p (u t) -> p u t", u=2)

    def k_proj(p):
        """K projection for pair p -> kt_tmp fp8 [128, 2048], then dup DMA."""
        kt_tmp = kttmp_pool.tile([128, 2048], FP8, tag="kttmp", name="kttmp")
        for quarter in range(4):
            pp = ps_pp.tile([128, 512], F32, tag="pp", name="kp")
            for tsub in range(4):
                tb = quarter * 4 + tsub
                for k in range(4):
                    nc.tensor.matmul(
                        pp[:, ts(tsub, 128)],
                        wk_sb[:, p * 1024 + k * 256: p * 1024 + k * 256 + 256]
                        .rearrange("p (u m) -> p u m", u=2),
                        hbt_pair(k, tb),
                        start=(k == 0), stop=(k == 3), perf_mode=DR)
            if sched.pick(612.0, 658.0) == "a":
                nc.scalar.activation(kt_tmp[:, ts(quarter, 512)], pp[:], AF.Copy)
            else:
                nc.vector.tensor_copy(kt_tmp[:, ts(quarter, 512)], pp[:])
        kt2 = kt2_pool.tile([128, 4096], FP8, tag="kt2", name="kt2")
        dst = kt2[:].rearrange("p (b u t) -> p b u t", b=16, u=2)
        src = kt_tmp[:].rearrange("p (b t) -> p b t", b=16)
        nc.sync.dma_start(dst[:, :, 0, :], src)
        nc.sync.dma_start(dst[:, :, 1, :], src)
        return kt2

    def q_proj(p):
        """Q projection for pair p (local 1024 tokens) -> qt fp8 [128, 1024]."""
        qt = qt_pool.tile([128, 1024], FP8, tag="qt", name="qt")
        for half in range(2):
            pp = ps_pp.tile([128, 512], F32, tag="pp", name="qp")
            for tsub in range(4):
                tb = half * 4 + tsub
                for k in range(4):
                    nc.tensor.matmul(
                        pp[:, ts(tsub, 128)],
                        wq_sb[:, p * 1024 + k * 256: p * 1024 + k * 256 + 256]
                        .rearrange("p (u m) -> p u m", u=2),
                        hbt_pair(k, tb),
                        start=(k == 0), stop=(k == 3), perf_mode=DR)
            if sched.pick(612.0, 658.0) == "a":
                nc.scalar.activation(qt[:, ts(half, 512)], pp[:], AF.Copy)
            else:
                nc.vector.tensor_copy(qt[:, ts(half, 512)], pp[:])
        return qt

    # ---- V projection: all 16 heads, out [tok, col] ----
    kq0 = {}
    for tb in range(JT):
        if tb == 2:
            emit_weight_dmas()
        if tb == 8:
            kq0["kt2"] = k_proj(0)
        if tb == 10:
            g_ap, b_ap = gamma_d.ap(), beta_d.ap()
            nc.gpsimd.dma_start(
                out=gamma_b[:],
                in_=bass.AP(tensor=g_ap.tensor, offset=g_ap.offset,
                            ap=[[0, 128], [1, D_MODEL]]))
            nc.gpsimd.dma_start(
                out=beta_b[:],
                in_=bass.AP(tensor=b_ap.tensor, offset=b_ap.offset,
                            ap=[[0, 128], [1, D_MODEL]]))
        if tb == 12:
            kq0["qt"] = q_proj(0)
        pp = ps_s2.tile([128, 1024], F32, tag="s2", name="vp")
        for cg in range(2):
            for k in range(4):
                nc.tensor.matmul(
                    pp[:, ts(cg, 512)],
                    hbt_pair(k, tb),
                    wv_sb[:, 2 * k * 1024: 2 * k * 1024 + 2048]
                    .rearrange("p (u n) -> p u n", u=2)
                    [:, :, cg * 512:(cg + 1) * 512],
                    start=(k == 0), stop=(k == 3), perf_mode=DR)
        # one strided copy: 16 heads' 64-col blocks -> v_all block tb
        dst = v_all[:].rearrange("p (hd b c) -> p hd b c",
                                 hd=N_HEAD, c=128)[:, :, tb, 0:64]
        src = pp[:].rearrange("p (hd c) -> p hd c", c=64)
        if sched.pick() == "a":
            nc.scalar.activation(dst, src, AF.Copy)
        else:
            nc.vector.tensor_copy(dst, src)


    at_tiles = {}   # itile -> attnT tile [128, 4096]

    def attn_scores(p, kt2, qt, itile, h, hook=None):
        """Scores + softmax-exp for (pair p, head h, 512-token itile)."""
        n = 2 * p + h
        e_aps = []
        for g in range(JG):
            s2 = ps_s2.tile([128, 1024], F32, tag="s2", name="s2")
            for u in range(2):
                jc = 2 * g + u
                nc.tensor.matmul(
                    s2[:, ts(u, 512)],
                    kt2[ts(h, 64), jc * 256: jc * 256 + 256]
                    .rearrange("p (u2 t) -> p u2 t", u2=2),
                    qt[ts(h, 64), ts(itile, 512)]
                    .unsqueeze(1).broadcast_to([64, 2, 512]),
                    start=True, stop=True, perf_mode=DR)
            if sched.pick() == "a":
                e_t = e_pool.tile([128, 1024], FP8, tag="e", name="e")
                nc.scalar.activation(e_t[:], s2[:], AF.Exp,
                                     bias=ebias[:], scale=EXP_SCALE)
                e_ap = e_t[:]
            else:
                e_t = e_pool.tile([128, 1024], U8, tag="e", name="e")
                nc.vector.tensor_scalar(e_t[:], s2[:], SCHD_A, SCHD_B,
                                        op0=ALU.mult, op1=ALU.add)
                e_ap = e_t[:].bitcast(FP8)
            e_aps.append(e_ap)
            if hook is not None and g == 3:
                hook()
                hook = None
        if hook is not None:
            hook()
        return (n, itile, h, e_aps)

    def attn_pv(ctx):
        """PV + normalization for a stream emitted by attn_scores."""
        n, itile, h, e_aps = ctx
        p = n // 2
        acc = ps_acc.tile([128, 512], F32, tag="acc", name="acc")
        for g in range(JG):
            nc.tensor.matmul(
                acc[:],
                v_all[:, n * 2048 + g * 256: n * 2048 + g * 256 + 256]
                .rearrange("p (u m) -> p u m", u=2),
                e_aps[g].rearrange("p (u n) -> p u n", u=2),
                start=(g == 0), stop=(g == JG - 1), perf_mode=DR)
        rec = sm_pool.tile([1, 512], F32, tag="rec", name="rec")
        nc.vector.reciprocal(rec[:], acc[64:65, :])
        sched.dve_extra(660.0)
        rb = sm_pool.tile([64, 512], F32, tag="rb", name="rb")
        nc.gpsimd.partition_broadcast(rb[:], rec[:])
        at_t = at_tiles[itile]
        dst = at_t[ts(h, 64), :].rearrange(
            "p (b q t) -> p b q t", b=4, q=8)[:, :, p, :]
        nc.vector.tensor_tensor(
            dst,
            acc[0:64, :].rearrange("p (b t) -> p b t", b=4),
            rb[:].rearrange("p (b t) -> p b t", b=4),
            op=ALU.mult)
        sched.dve_extra(660.0)

    wo_state = {}

    def wo_mats(itile):
        """Output projection + residual + LN stats for a 512-token block."""
        at_t = at_tiles[itile]
        xs = []
        # mv cols (2*tb4, 2*tb4+1) = (mean, var) per 128-token sub-block
        mv = sm_pool.tile([128, 8], F32, tag="mv", name="mv")
        hbrs = []
        for tb4 in range(4):
            isub = itile * 4 + tb4
            hbr = hbr_pool.tile([128, D_MODEL], F32, tag="hbr", name="hbr")
            nc.gpsimd.dma_start(hbr[:], hbres_d[ts(isub, 128), :])
            hbrs.append(hbr)
        for tb4 in range(4):
            isub = itile * 4 + tb4
            hbr = hbrs[tb4]
            x = x_pool.tile([128, D_MODEL], F32, tag="x", name="x")
            if itile == 1:
                # attention is over: the wide s2 psum tiles are free
                pp = ps_s2.tile([128, 1024], F32, tag="s2", name="op")
                for dm in range(2):
                    for q in range(4):
                        nc.tensor.matmul(
                            pp[:, ts(dm, 512)],
                            at_t[:, tb4 * 1024 + 2 * q * 128:
                                 tb4 * 1024 + 2 * q * 128 + 256]
                            .rearrange("p (u m) -> p u m", u=2),
                            wo_sb[:, 2 * q * 1024: 2 * q * 1024 + 2048]
                            .rearrange("p (u n) -> p u n", u=2)
                            [:, :, dm * 512:(dm + 1) * 512],
                            start=(q == 0), stop=(q == 3), perf_mode=DR)
                nc.vector.scalar_tensor_tensor(
                    x[:], pp[:], OUT_DESCALE, hbr[:],
                    op0=ALU.mult, op1=ALU.add)
                sched.dve_extra(1200.0)
            else:
                for dm in range(2):
                    pp = ps_pp.tile([128, 512], F32, tag="pp", name="op")
                    for q in range(4):
                        nc.tensor.matmul(
                            pp[:],
                            at_t[:, tb4 * 1024 + 2 * q * 128:
                                 tb4 * 1024 + 2 * q * 128 + 256]
                            .rearrange("p (u m) -> p u m", u=2),
                            wo_sb[:, 2 * q * 1024: 2 * q * 1024 + 2048]
                            .rearrange("p (u n) -> p u n", u=2)
                            [:, :, dm * 512:(dm + 1) * 512],
                            start=(q == 0), stop=(q == 3), perf_mode=DR)
                    nc.vector.scalar_tensor_tensor(
                        x[:, ts(dm, 512)], pp[:], OUT_DESCALE,
                        hbr[:, ts(dm, 512)],
                        op0=ALU.mult, op1=ALU.add)
                    sched.dve_extra(660.0)
            xs.append(x)
            # LN stats on DVE (bn_stats/bn_aggr -> mean, var)
            stats = sm_pool.tile([128, 2, 6], F32, tag="bst", name="bst")
            for g2 in range(2):
                nc.vector.bn_stats(stats[:, g2, :], x[:, ts(g2, 512)])
            nc.vector.bn_aggr(mv[:, 2 * tb4:2 * tb4 + 2], stats[:])
            sched.dve_extra(1450.0)
        wo_state[itile] = (xs, mv)

    def wo_tail(itile):
        """rstd + normalize + gamma/beta + output DMA."""
        xs, mv = wo_state.pop(itile)
        var_v = mv[:].rearrange("p (b t) -> p b t", t=2)[:, :, 1:2].squeeze(2)
        rstd = sm_pool.tile([128, 4], F32, tag="rstd", name="rstd")
        rvar = sm_pool.tile([128, 4], F32, tag="rvar", name="rvar")
        nc.vector.tensor_scalar(rvar[:], var_v, 1.0, EPS,
                                op0=ALU.mult, op1=ALU.add)
        nc.vector.reciprocal(rstd[:], rvar[:])
        nc.scalar.activation(rstd[:], rstd[:], AF.Sqrt)
        sched.act_extra(400.0)
        for tb4 in range(4):
            isub = itile * 4 + tb4
            x = xs[tb4]
            # fused LN tail on DVE: t = (x - mu) * gamma ; y = t * rstd + beta
            nc.vector.scalar_tensor_tensor(
                x[:], x[:], mv[:, 2 * tb4:2 * tb4 + 1], gamma_b[:],
                op0=ALU.subtract, op1=ALU.mult)
            nc.vector.scalar_tensor_tensor(
                x[:], x[:], rstd[:, tb4:tb4 + 1], beta_b[:],
                op0=ALU.mult, op1=ALU.add)
            sched.dve_extra(1200.0)
            eng = nc.sync if tb4 % 2 == 0 else nc.gpsimd
            eng.dma_start(out_d[ts(isub, 128), :], x[:])

    # ---- main pair loop with cross-pair pipelining ----
    kt2 = kq0["kt2"]
    qt = kq0["qt"]
    at_tiles[0] = at_pool.tile([128, 4096], FP8, tag="at0", name="at0")
    at_tiles[1] = at_pool.tile([128, 4096], FP8, tag="at1", name="at1")

    next_ref = {}

    if taps is not None:
        nc.sync.dma_start(taps["t_kt2"][:, :], kt2[:])
        nc.sync.dma_start(taps["t_qt"][:, :], qt[:])
        nc.sync.dma_start(taps["t_vall"][:, :], v_all[:, 0:2048])

    pending = []
    for p in range(N_PAIR):
        hooks = {}
        if p + 1 < N_PAIR:
            pn = p + 1
            hooks[(0, 0)] = lambda pn=pn: next_ref.__setitem__(
                "kt2", k_proj(pn))
            hooks[(1, 0)] = lambda pn=pn: next_ref.__setitem__(
                "qt", q_proj(pn))
        else:
            def _wo0_hook():
                while pending:
                    attn_pv(pending.pop(0))
                wo_mats(0)
            hooks[(1, 0)] = _wo0_hook
        for itile in range(2):
            for h in range(2):
                ctx = attn_scores(p, kt2, qt, itile, h,
                                  hook=hooks.pop((itile, h), None))
                pending.append(ctx)
                if len(pending) > 1:
                    attn_pv(pending.pop(0))
        if p + 1 < N_PAIR:
            kt2 = next_ref.pop("kt2")
            qt = next_ref.pop("qt")
    while pending:
        attn_pv(pending.pop(0))
    wo_tail(0)
    wo_mats(1)
    wo_tail(1)
    if taps is not None:
        nc.sync.dma_start(taps["t_at"][:, :], at_tiles[0][:])


_program_cache = {}


def _get_program():
    if "nc" not in _program_cache:
        _program_cache["nc"] = build_program()
    return _program_cache["nc"]


def _q8(x):
    return np.ascontiguousarray(np.asarray(x, np.float32).astype(E4))


def _shard_inputs(h, Wq, Wkv, Wo, gamma, beta):
    """Build the 8 per-core input maps (host-side numpy only)."""
    h = np.asarray(h, np.float32)
    Wq = np.asarray(Wq, np.float32) * WS
    Wkv = np.asarray(Wkv, np.float32) * WS
    Wo = np.asarray(Wo, np.float32) * WS
    gamma = np.asarray(gamma, np.float32)
    beta = np.asarray(beta, np.float32)

    Wk = Wkv[:, :N_HEAD * D_HEAD]
    Wv = Wkv[:, N_HEAD * D_HEAD:]

    def pack_qk(W):
        # w8[p, cb*1024 + k*256 + u*128 + col] = W[k*256+u*128+p, cb*128+col]
        B = _q8(W).reshape(4, 2, 128, 8, 128)        # [k, u, p, cb, col]
        return np.ascontiguousarray(
            B.transpose(2, 3, 0, 1, 4).reshape(128, 8192))

    def pack_rowmajor(W):
        # w8[p, c*1024 + col] = W[128c+p, col]
        C = _q8(W).reshape(8, 128, 1024)             # [c, p, col]
        return np.ascontiguousarray(C.transpose(1, 0, 2).reshape(128, 8192))

    wq8 = pack_qk(Wq)
    wk8 = pack_qk(Wk)
    wv8 = pack_rowmajor(Wv)
    wo8 = pack_rowmajor(Wo)

    in_maps = []
    for core in range(N_CORES):
        b, r = divmod(core, 2)
        hb_full = h[:, b, :]
        if r == 0:
            hb_perm = hb_full
        else:
            hb_perm = np.concatenate([hb_full[LOCAL:], hb_full[:LOCAL]], 0)
        # hbt8[p, k*4096 + tb*256 + u*128 + t] = hb_perm[tb*128+t, 256k+128u+p]
        A = _q8(hb_perm).reshape(16, 128, 4, 2, 128)  # [tb, t, k, u, p]
        hbt8 = np.ascontiguousarray(
            A.transpose(4, 2, 0, 3, 1).reshape(128, 16384))
        in_maps.append({
            "hbt": hbt8,
            "wq": wq8, "wk": wk8, "wv": wv8, "wo": wo8,
            "hbres": np.ascontiguousarray(hb_perm[:LOCAL]),
            "gamma": gamma, "beta": beta,
        })
    return in_maps


def kernel(h, Wq, Wkv, Wo, gamma, beta, _trace=False):
    nc = _get_program()
    in_maps = _shard_inputs(h, Wq, Wkv, Wo, gamma, beta)
    res = run_bass_kernel_spmd(nc, in_maps, list(range(N_CORES)), trace=_trace)
    if _trace:
        kernel.last_results = res

    out = np.empty((SEQ, BATCH, D_MODEL), np.float32)
    for core in range(N_CORES):
        b, r = divmod(core, 2)
        out[r * LOCAL:(r + 1) * LOCAL, b, :] = res.results[core]["out"]
    return out


# revision 43
# speedup vs baseline: 1.0222x; 1.0222x over previous
"""Trainium2 Bass kernel for nn_MultiHeadAttn_80126909874682 (v3, fp8 DoubleRow).

Full MHA layer: QKV projection -> 16-head attention (seq 2048) -> output
projection -> residual -> LayerNorm, over h [2048, 4, 1024] fp32.

Sharding (8 NeuronCores, zero collectives):
  core c -> batch b = c // 2, token-half r = c % 2.
  Each core computes K/V for all 2048 tokens of its batch (all 16 heads)
  and Q / attention / output projection / LayerNorm for its 1024 local
  tokens.  The per-core inputs are permuted so local tokens come first.

v3: every matmul runs as an fp8e4 DoubleRow matmul (two 128-row k-tiles
per instruction, 0.5 PE cycles per output row).  Weights and h are
pre-quantized to fp8e4 on the host with a x16 / x1 scale; score descale
(1/2048) and a -5*ln2 bias are folded into the softmax exp; the final
1/256 descale is folded into the residual add.  Softmax exp is split
between the ACT engine (exact exp, fp8 output) and the DVE engine
(one-instruction Schraudolph exp: scores * A + B rounded to uint8 IS
the fp8e4 bit pattern).  Denominators come from a ones column packed
into the PV weight tiles; LayerNorm statistics and tail run on Pool.

DoubleRow ISA constraint honored throughout: the stationary operand
must be [p, 2, 128] with the two 128-column slots contiguous (kt tiles
are duplicated via SBUF->SBUF DMA to satisfy this); the moving operand
tolerates arbitrary slot strides including 0 (broadcast).
"""

import os
import sys

os.environ.setdefault("JAX_PLATFORMS", "axon")
sys.path.insert(0, "/opt/trn_rl_repo")

import numpy as np
import ml_dtypes

import concourse.bass as bass
import concourse.tile as tile
from concourse import bacc, mybir
from concourse.bass import ts
from concourse.bass_utils import run_bass_kernel_spmd

N_HEAD = 16
D_MODEL = 1024
D_HEAD = 64
SEQ = 2048
BATCH = 4
EPS = 1e-5
N_CORES = 8

LOCAL = SEQ // 2            # tokens owned per core (1024)
N_PAIR = N_HEAD // 2        # head pairs (8)
JT = SEQ // 128             # j tiles (16)
JG = JT // 2                # j tile pairs (8)

F32 = mybir.dt.float32
FP8 = mybir.dt.float8e4
U8 = mybir.dt.uint8
AF = mybir.ActivationFunctionType
ALU = mybir.AluOpType
DR = mybir.MatmulPerfMode.DoubleRow
E4 = ml_dtypes.float8_e4m3

LOG2E = 1.4426950408889634
WS = 16.0                       # weight quantization scale
EXP_SCALE = 1.0 / 4096.0        # score descale (16*16*8 * 2 dup slots)
EXP_BIAS = -5.0 * float(np.log(2.0))
SCHD_A = 8.0 * LOG2E * EXP_SCALE
SCHD_B = 56.0 + 8.0 * LOG2E * EXP_BIAS   # = 16.0
OUT_DESCALE = 1.0 / 256.0

# drain scheduling: pick the engine whose accumulated queue cost is lowest,
# so consecutive exp tiles alternate ACT/DVE and run concurrently.
ACT_EXP_NS, DVE_EXP_NS = 1038.0, 1330.0
ACT_CPY_NS, DVE_CPY_NS = 1038.0, 1192.0


class _Sched:
    """Cost-balancing chooser between ACT ('a') and DVE ('d')."""

    def __init__(self):
        self.a_ns = 0.0
        self.d_ns = 0.0   # DVE-only tail work is accounted as emitted

    def pick(self, a_cost=ACT_EXP_NS, d_cost=DVE_EXP_NS):
        if self.a_ns + a_cost <= self.d_ns + d_cost:
            self.a_ns += a_cost
            return "a"
        self.d_ns += d_cost
        return "d"

    def dve_extra(self, ns):
        self.d_ns += ns

    def act_extra(self, ns):
        self.a_ns += ns


DEBUG_TAPS = False


def build_program():
    nc = bacc.Bacc()

    hbt = nc.declare_dram_parameter("hbt", [128, 16384], FP8, isOutput=False)
    wq = nc.declare_dram_parameter("wq", [128, 8192], FP8, isOutput=False)
    wk = nc.declare_dram_parameter("wk", [128, 8192], FP8, isOutput=False)
    wv = nc.declare_dram_parameter("wv", [128, 8192], FP8, isOutput=False)
    wo = nc.declare_dram_parameter("wo", [128, 8192], FP8, isOutput=False)
    hbres = nc.declare_dram_parameter("hbres", [LOCAL, D_MODEL], F32,
                                      isOutput=False)
    gamma = nc.declare_dram_parameter("gamma", [D_MODEL], F32, isOutput=False)
    beta = nc.declare_dram_parameter("beta", [D_MODEL], F32, isOutput=False)
    out = nc.declare_dram_parameter("out", [LOCAL, D_MODEL], F32, isOutput=True)
    taps = None
    if DEBUG_TAPS:
        taps = {
            "t_kt2": nc.declare_dram_parameter("t_kt2", [128, 4096], FP8, isOutput=True),
            "t_qt": nc.declare_dram_parameter("t_qt", [128, 1024], FP8, isOutput=True),
            "t_e": nc.declare_dram_parameter("t_e", [128, 1024], FP8, isOutput=True),
            "t_vall": nc.declare_dram_parameter("t_vall", [128, 2048], FP8, isOutput=True),
            "t_at": nc.declare_dram_parameter("t_at", [128, 4096], FP8, isOutput=True),
            "t_rb": nc.declare_dram_parameter("t_rb", [64, 512], F32, isOutput=True),
            "t_acc": nc.declare_dram_parameter("t_acc", [16, 512], F32, isOutput=True),
            "t_rec": nc.declare_dram_parameter("t_rec", [1, 512], F32, isOutput=True),
        }

    with tile.TileContext(nc) as tc:
        with (
            tc.tile_pool(name="consts", bufs=1) as consts,
            tc.tile_pool(name="hbt", bufs=1) as hbt_pool,
            tc.tile_pool(name="wqk", bufs=1) as wqk_pool,
            tc.tile_pool(name="wvo", bufs=1) as wvo_pool,
            tc.tile_pool(name="vall", bufs=1) as v_pool,
            tc.tile_pool(name="kt2", bufs=2) as kt2_pool,
            tc.tile_pool(name="kttmp", bufs=2) as kttmp_pool,
            tc.tile_pool(name="qt", bufs=2) as qt_pool,
            tc.tile_pool(name="e", bufs=12) as e_pool,
            tc.tile_pool(name="attnT", bufs=2) as at_pool,
            tc.tile_pool(name="x", bufs=4) as x_pool,
            tc.tile_pool(name="hbr", bufs=4) as hbr_pool,
            tc.tile_pool(name="small", bufs=4) as sm_pool,
            tc.tile_pool(name="ps_s2", bufs=3, space="PSUM") as ps_s2,
            tc.tile_pool(name="ps_acc", bufs=1, space="PSUM") as ps_acc,
            tc.tile_pool(name="ps_pp", bufs=1, space="PSUM") as ps_pp,
        ):
            _emit(nc, tc, hbt, wq, wk, wv, wo, hbres, gamma, beta, out,
                  consts, hbt_pool, wqk_pool, wvo_pool, v_pool, kt2_pool,
                  kttmp_pool, qt_pool, e_pool, at_pool, x_pool, hbr_pool,
                  sm_pool, ps_s2, ps_acc, ps_pp, taps)

    nc.finalize()
    return nc


def _emit(nc, tc, hbt_d, wq_d, wk_d, wv_d, wo_d, hbres_d, gamma_d, beta_d,
          out_d, consts, hbt_pool, wqk_pool, wvo_pool, v_pool, kt2_pool,
          kttmp_pool, qt_pool, e_pool, at_pool, x_pool, hbr_pool, sm_pool,
          ps_s2, ps_acc, ps_pp, taps=None):
    sched = _Sched()

    # ---- constants ----
    gamma_b = consts.tile([128, D_MODEL], F32, name="gamma_b")
    beta_b = consts.tile([128, D_MODEL], F32, name="beta_b")
    ebias = consts.tile([128, 1], F32, name="ebias")
    eps_t = consts.tile([128, 1], F32, name="eps")
    nc.vector.memset(ebias[:], EXP_BIAS)
    nc.vector.memset(eps_t[:], EPS)
    # ---- weight / activation DMAs (wv + hbt first: V-proj gating) ----
    hbt = hbt_pool.tile([128, 16384], FP8, name="hbt")
    wq_sb = wqk_pool.tile([128, 8192], FP8, name="wq")
    wk_sb = wqk_pool.tile([128, 8192], FP8, name="wk")
    wv_sb = wvo_pool.tile([128, 8192], FP8, name="wv")
    wo_sb = wvo_pool.tile([128, 8192], FP8, name="wo")
    for c in range(4):
        eng = nc.sync if c % 2 == 0 else nc.gpsimd
        eng.dma_start(wv_sb[:, ts(c, 2048)], wv_d[:, ts(c, 2048)])
    for half in range(2):
        for k in range(4):
            off = k * 4096 + half * 2048
            eng = nc.sync if k % 2 == 0 else nc.gpsimd
            eng.dma_start(hbt[:, off:off + 2048], hbt_d[:, off:off + 2048])
    def emit_weight_dmas():
        for c in range(4):
            eng = nc.sync if c % 2 == 0 else nc.gpsimd
            eng.dma_start(wk_sb[:, ts(c, 2048)], wk_d[:, ts(c, 2048)])
            eng.dma_start(wq_sb[:, ts(c, 2048)], wq_d[:, ts(c, 2048)])
            eng.dma_start(wo_sb[:, ts(c, 2048)], wo_d[:, ts(c, 2048)])

    # ---- v_all: [128 j, 16 heads * 8 jpairs * 256]; per 128-block:
    #      cols 0:64 = 16*v, col 64 = 1.0 (denominator), 65:128 = 0 ----
    v_all = v_pool.tile([128, N_HEAD * 2048], FP8, name="v_all")
    va = v_all[:].rearrange("p (b c) -> p b c", c=128)     # [128, 256, 128]
    nc.gpsimd.memset(va[:, :, 64:65], 1.0)
    nc.gpsimd.memset(va[:, :, 65:128], 0.0)

    def hbt_pair(k, tb):
        """hbt [p, 2, 128] weights view for dm-chunk pair k, token block tb."""
        base = k * 4096 + tb * 256
        return hbt[:, base:base + 256].rearrange("p (u t) -> p u t", u=2)

    def k_proj(p):
        """K projection for pair p -> kt_tmp fp8 [128, 2048], then dup DMA."""
        kt_tmp = kttmp_pool.tile([128, 2048], FP8, tag="kttmp", name="kttmp")
        for quarter in range(4):
            pp = ps_pp.tile([128, 512], F32, tag="pp", name="kp")
            for tsub in range(4):
                tb = quarter * 4 + tsub
                for k in range(4):
                    nc.tensor.matmul(
                        pp[:, ts(tsub, 128)],
                        wk_sb[:, p * 1024 + k * 256: p * 1024 + k * 256 + 256]
                        .rearrange("p (u m) -> p u m", u=2),
                        hbt_pair(k, tb),
                        start=(k == 0), stop=(k == 3), perf_mode=DR)
            if sched.pick(612.0, 658.0) == "a":
                nc.scalar.activation(kt_tmp[:, ts(quarter, 512)], pp[:], AF.Copy)
            else:
                nc.vector.tensor_copy(kt_tmp[:, ts(quarter, 512)], pp[:])
        kt2 = kt2_pool.tile([128, 4096], FP8, tag="kt2", name="kt2")
        dst = kt2[:].rearrange("p (b u t) -> p b u t", b=16, u=2)
        src = kt_tmp[:].rearrange("p (b t) -> p b t", b=16)
        nc.sync.dma_start(dst[:, :, 0, :], src)
        nc.sync.dma_start(dst[:, :, 1, :], src)
        return kt2

    def q_proj(p):
        """Q projection for pair p (local 1024 tokens) -> qt fp8 [128, 1024]."""
        qt = qt_pool.tile([128, 1024], FP8, tag="qt", name="qt")
        for half in range(2):
            pp = ps_pp.tile([128, 512], F32, tag="pp", name="qp")
            for tsub in range(4):
                tb = half * 4 + tsub
                for k in range(4):
                    nc.tensor.matmul(
                        pp[:, ts(tsub, 128)],
                        wq_sb[:, p * 1024 + k * 256: p * 1024 + k * 256 + 256]
                        .rearrange("p (u m) -> p u m", u=2),
                        hbt_pair(k, tb),
                        start=(k == 0), stop=(k == 3), perf_mode=DR)
            if sched.pick(612.0, 658.0) == "a":
                nc.scalar.activation(qt[:, ts(half, 512)], pp[:], AF.Copy)
            else:
                nc.vector.tensor_copy(qt[:, ts(half, 512)], pp[:])
        return qt

    # ---- V projection: all 16 heads, out [tok, col] ----
    kq0 = {}
    for tb in range(JT):
        if tb == 2:
            emit_weight_dmas()
        if tb == 8:
            kq0["kt2"] = k_proj(0)
        if tb == 10:
            g_ap, b_ap = gamma_d.ap(), beta_d.ap()
            nc.gpsimd.dma_start(
                out=gamma_b[:],
                in_=bass.AP(tensor=g_ap.tensor, offset=g_ap.offset,
                            ap=[[0, 128], [1, D_MODEL]]))
            nc.gpsimd.dma_start(
                out=beta_b[:],
                in_=bass.AP(tensor=b_ap.tensor, offset=b_ap.offset,
                            ap=[[0, 128], [1, D_MODEL]]))
        if tb == 12:
            kq0["qt"] = q_proj(0)
        pp = ps_s2.tile([128, 1024], F32, tag="s2", name="vp")
        for cg in range(2):
            for k in range(4):
                nc.tensor.matmul(
                    pp[:, ts(cg, 512)],
                    hbt_pair(k, tb),
                    wv_sb[:, 2 * k * 1024: 2 * k * 1024 + 2048]
                    .rearrange("p (u n) -> p u n", u=2)
                    [:, :, cg * 512:(cg + 1) * 512],
                    start=(k == 0), stop=(k == 3), perf_mode=DR)
        # one strided copy: 16 heads' 64-col blocks -> v_all block tb
        dst = v_all[:].rearrange("p (hd b c) -> p hd b c",
                                 hd=N_HEAD, c=128)[:, :, tb, 0:64]
        src = pp[:].rearrange("p (hd c) -> p hd c", c=64)
        if sched.pick() == "a":
            nc.scalar.activation(dst, src, AF.Copy)
        else:
            nc.vector.tensor_copy(dst, src)


    at_tiles = {}   # itile -> attnT tile [128, 4096]

    def attn_scores(p, kt2, qt, itile, h, hook=None):
        """Scores + softmax-exp for (pair p, head h, 512-token itile)."""
        n = 2 * p + h
        e_aps = []
        for g in range(JG):
            s2 = ps_s2.tile([128, 1024], F32, tag="s2", name="s2")
            for u in range(2):
                jc = 2 * g + u
                nc.tensor.matmul(
                    s2[:, ts(u, 512)],
                    kt2[ts(h, 64), jc * 256: jc * 256 + 256]
                    .rearrange("p (u2 t) -> p u2 t", u2=2),
                    qt[ts(h, 64), ts(itile, 512)]
                    .unsqueeze(1).broadcast_to([64, 2, 512]),
                    start=True, stop=True, perf_mode=DR)
            if sched.pick() == "a":
                e_t = e_pool.tile([128, 1024], FP8, tag="e", name="e")
                nc.scalar.activation(e_t[:], s2[:], AF.Exp,
                                     bias=ebias[:], scale=EXP_SCALE)
                e_ap = e_t[:]
            else:
                e_t = e_pool.tile([128, 1024], U8, tag="e", name="e")
                nc.vector.tensor_scalar(e_t[:], s2[:], SCHD_A, SCHD_B,
                                        op0=ALU.mult, op1=ALU.add)
                e_ap = e_t[:].bitcast(FP8)
            e_aps.append(e_ap)
            if hook is not None and g == 3:
                hook()
                hook = None
        if hook is not None:
            hook()
        return (n, itile, h, e_aps)

    def attn_pv(ctx):
        """PV + normalization for a stream emitted by attn_scores."""
        n, itile, h, e_aps = ctx
        p = n // 2
        acc = ps_acc.tile([128, 512], F32, tag="acc", name="acc")
        for g in range(JG):
            nc.tensor.matmul(
                acc[:],
                v_all[:, n * 2048 + g * 256: n * 2048 + g * 256 + 256]
                .rearrange("p (u m) -> p u m", u=2),
                e_aps[g].rearrange("p (u n) -> p u n", u=2),
                start=(g == 0), stop=(g == JG - 1), perf_mode=DR)
        rec = sm_pool.tile([1, 512], F32, tag="rec", name="rec")
        nc.vector.reciprocal(rec[:], acc[64:65, :])
        sched.dve_extra(660.0)
        rb = sm_pool.tile([64, 512], F32, tag="rb", name="rb")
        nc.gpsimd.partition_broadcast(rb[:], rec[:])
        at_t = at_tiles[itile]
        dst = at_t[ts(h, 64), :].rearrange(
            "p (b q t) -> p b q t", b=4, q=8)[:, :, p, :]
        nc.vector.tensor_tensor(
            dst,
            acc[0:64, :].rearrange("p (b t) -> p b t", b=4),
            rb[:].rearrange("p (b t) -> p b t", b=4),
            op=ALU.mult)
        sched.dve_extra(660.0)

    wo_state = {}

    def wo_mats(itile):
        """Output projection + residual + LN stats for a 512-token block."""
        at_t = at_tiles[itile]
        xs = []
        # mv cols (2*tb4, 2*tb4+1) = (mean, var) per 128-token sub-block
        mv = sm_pool.tile([128, 8], F32, tag="mv", name="mv")
        hbrs = []
        for tb4 in range(4):
            isub = itile * 4 + tb4
            hbr = hbr_pool.tile([128, D_MODEL], F32, tag="hbr", name="hbr")
            nc.gpsimd.dma_start(hbr[:], hbres_d[ts(isub, 128), :])
            hbrs.append(hbr)
        for tb4 in range(4):
            isub = itile * 4 + tb4
            hbr = hbrs[tb4]
            x = x_pool.tile([128, D_MODEL], F32, tag="x", name="x")
            if itile == 1:
                # attention is over: the wide s2 psum tiles are free
                pp = ps_s2.tile([128, 1024], F32, tag="s2", name="op")
                for dm in range(2):
                    for q in range(4):
                        nc.tensor.matmul(
                            pp[:, ts(dm, 512)],
                            at_t[:, tb4 * 1024 + 2 * q * 128:
                                 tb4 * 1024 + 2 * q * 128 + 256]
                            .rearrange("p (u m) -> p u m", u=2),
                            wo_sb[:, 2 * q * 1024: 2 * q * 1024 + 2048]
                            .rearrange("p (u n) -> p u n", u=2)
                            [:, :, dm * 512:(dm + 1) * 512],
                            start=(q == 0), stop=(q == 3), perf_mode=DR)
                nc.vector.scalar_tensor_tensor(
                    x[:], pp[:], OUT_DESCALE, hbr[:],
                    op0=ALU.mult, op1=ALU.add)
                sched.dve_extra(1200.0)
            else:
                for dm in range(2):
                    pp = ps_pp.tile([128, 512], F32, tag="pp", name="op")
                    for q in range(4):
                        nc.tensor.matmul(
                            pp[:],
                            at_t[:, tb4 * 1024 + 2 * q * 128:
                                 tb4 * 1024 + 2 * q * 128 + 256]
                            .rearrange("p (u m) -> p u m", u=2),
                            wo_sb[:, 2 * q * 1024: 2 * q * 1024 + 2048]
                            .rearrange("p (u n) -> p u n", u=2)
                            [:, :, dm * 512:(dm + 1) * 512],
                            start=(q == 0), stop=(q == 3), perf_mode=DR)
                    nc.vector.scalar_tensor_tensor(
                        x[:, ts(dm, 512)], pp[:], OUT_DESCALE,
                        hbr[:, ts(dm, 512)],
                        op0=ALU.mult, op1=ALU.add)
                    sched.dve_extra(660.0)
            xs.append(x)
            # LN stats on DVE (bn_stats/bn_aggr -> mean, var)
            stats = sm_pool.tile([128, 2, 6], F32, tag="bst", name="bst")
            for g2 in range(2):
                nc.vector.bn_stats(stats[:, g2, :], x[:, ts(g2, 512)])
            nc.vector.bn_aggr(mv[:, 2 * tb4:2 * tb4 + 2], stats[:])
            sched.dve_extra(1450.0)
        wo_state[itile] = (xs, mv)

    def wo_tail(itile):
        """rstd + normalize + gamma/beta + output DMA."""
        xs, mv = wo_state.pop(itile)
        var_v = mv[:].rearrange("p (b t) -> p b t", t=2)[:, :, 1:2].squeeze(2)
        rstd = sm_pool.tile([128, 4], F32, tag="rstd", name="rstd")
        rvar = sm_pool.tile([128, 4], F32, tag="rvar", name="rvar")
        nc.vector.tensor_scalar(rvar[:], var_v, 1.0, EPS,
                                op0=ALU.mult, op1=ALU.add)
        nc.vector.reciprocal(rstd[:], rvar[:])
        nc.scalar.activation(rstd[:], rstd[:], AF.Sqrt)
        sched.act_extra(400.0)
        for tb4 in range(4):
            isub = itile * 4 + tb4
            x = xs[tb4]
            # fused LN tail on DVE: t = (x - mu) * gamma ; y = t * rstd + beta
            nc.vector.scalar_tensor_tensor(
                x[:], x[:], mv[:, 2 * tb4:2 * tb4 + 1], gamma_b[:],
                op0=ALU.subtract, op1=ALU.mult)
            nc.vector.scalar_tensor_tensor(
                x[:], x[:], rstd[:, tb4:tb4 + 1], beta_b[:],
                op0=ALU.mult, op1=ALU.add)
            sched.dve_extra(1200.0)
            eng = nc.sync if tb4 % 2 == 0 else nc.gpsimd
            eng.dma_start(out_d[ts(isub, 128), :], x[:])

    # ---- main pair loop with cross-pair pipelining ----
    kt2 = kq0["kt2"]
    qt = kq0["qt"]
    at_tiles[0] = at_pool.tile([128, 4096], FP8, tag="at0", name="at0")
    at_tiles[1] = at_pool.tile([128, 4096], FP8, tag="at1", name="at1")

    next_ref = {}

    if taps is not None:
        nc.sync.dma_start(taps["t_kt2"][:, :], kt2[:])
        nc.sync.dma_start(taps["t_qt"][:, :], qt[:])
        nc.sync.dma_start(taps["t_vall"][:, :], v_all[:, 0:2048])

    pending = []
    for p in range(N_PAIR):
        hooks = {}
        if p + 1 < N_PAIR:
            pn = p + 1
            hooks[(0, 0)] = lambda pn=pn: next_ref.__setitem__(
                "kt2", k_proj(pn))
            hooks[(1, 0)] = lambda pn=pn: next_ref.__setitem__(
                "qt", q_proj(pn))
        else:
            def _wo0_hook():
                while pending:
                    attn_pv(pending.pop(0))
                wo_mats(0)
            hooks[(1, 0)] = _wo0_hook
        for itile in range(2):
            for h in range(2):
                ctx = attn_scores(p, kt2, qt, itile, h,
                                  hook=hooks.pop((itile, h), None))
                pending.append(ctx)
                if len(pending) > 1:
                    attn_pv(pending.pop(0))
        if p + 1 < N_PAIR:
            kt2 = next_ref.pop("kt2")
            qt = next_ref.pop("qt")
    while pending:
        attn_pv(pending.pop(0))
    wo_tail(0)
    wo_mats(1)
    wo_tail(1)
    if taps is not None:
        nc.sync.dma_start(taps["t_at"][:, :], at_tiles[0][:])


_program_cache = {}


def _get_program():
    if "nc" not in _program_cache:
        _program_cache["nc"] = build_program()
    return _program_cache["nc"]


def _q8(x):
    return np.ascontiguousarray(np.asarray(x, np.float32).astype(E4))


def _shard_inputs(h, Wq, Wkv, Wo, gamma, beta):
    """Build the 8 per-core input maps (host-side numpy only)."""
    h = np.asarray(h, np.float32)
    Wq = np.asarray(Wq, np.float32) * WS
    Wkv = np.asarray(Wkv, np.float32) * WS
    Wo = np.asarray(Wo, np.float32) * WS
    gamma = np.asarray(gamma, np.float32)
    beta = np.asarray(beta, np.float32)

    Wk = Wkv[:, :N_HEAD * D_HEAD]
    Wv = Wkv[:, N_HEAD * D_HEAD:]

    def pack_qk(W):
        # w8[p, cb*1024 + k*256 + u*128 + col] = W[k*256+u*128+p, cb*128+col]
        B = _q8(W).reshape(4, 2, 128, 8, 128)        # [k, u, p, cb, col]
        return np.ascontiguousarray(
            B.transpose(2, 3, 0, 1, 4).reshape(128, 8192))

    def pack_rowmajor(W):
        # w8[p, c*1024 + col] = W[128c+p, col]
        C = _q8(W).reshape(8, 128, 1024)             # [c, p, col]
        return np.ascontiguousarray(C.transpose(1, 0, 2).reshape(128, 8192))

    wq8 = pack_qk(Wq)
    wk8 = pack_qk(Wk)
    wv8 = pack_rowmajor(Wv)
    wo8 = pack_rowmajor(Wo)

    in_maps = []
    for core in range(N_CORES):
        b, r = divmod(core, 2)
        hb_full = h[:, b, :]
        if r == 0:
            hb_perm = hb_full
        else:
            hb_perm = np.concatenate([hb_full[LOCAL:], hb_full[:LOCAL]], 0)
        # hbt8[p, k*4096 + tb*256 + u*128 + t] = hb_perm[tb*128+t, 256k+128u+p]
        A = _q8(hb_perm).reshape(16, 128, 4, 2, 128)  # [tb, t, k, u, p]
        hbt8 = np.ascontiguousarray(
            A.transpose(4, 2, 0, 3, 1).reshape(128, 16384))
        in_maps.append({
            "hbt": hbt8,
            "wq": wq8, "wk": wk8, "wv": wv8, "wo": wo8,
            "hbres": np.ascontiguousarray(hb_perm[:LOCAL]),
            "gamma": gamma, "beta": beta,
        })
    return in_maps


def kernel(h, Wq, Wkv, Wo, gamma, beta, _trace=False):
    nc = _get_program()
    in_maps = _shard_inputs(h, Wq, Wkv, Wo, gamma, beta)
    res = run_bass_kernel_spmd(nc, in_maps, list(range(N_CORES)), trace=_trace)
    if _trace:
        kernel.last_results = res

    out = np.empty((SEQ, BATCH, D_MODEL), np.float32)
    for core in range(N_CORES):
        b, r = divmod(core, 2)
        out[r * LOCAL:(r + 1) * LOCAL, b, :] = res.results[core]["out"]
    return out


# revision 44
# speedup vs baseline: 1.0267x; 1.0044x over previous
"""Trainium2 Bass kernel for nn_MultiHeadAttn_80126909874682 (v3, fp8 DoubleRow).

Full MHA layer: QKV projection -> 16-head attention (seq 2048) -> output
projection -> residual -> LayerNorm, over h [2048, 4, 1024] fp32.

Sharding (8 NeuronCores, zero collectives):
  core c -> batch b = c // 2, token-half r = c % 2.
  Each core computes K/V for all 2048 tokens of its batch (all 16 heads)
  and Q / attention / output projection / LayerNorm for its 1024 local
  tokens.  The per-core inputs are permuted so local tokens come first.

v3: every matmul runs as an fp8e4 DoubleRow matmul (two 128-row k-tiles
per instruction, 0.5 PE cycles per output row).  Weights and h are
pre-quantized to fp8e4 on the host with a x16 / x1 scale; score descale
(1/2048) and a -5*ln2 bias are folded into the softmax exp; the final
1/256 descale is folded into the residual add.  Softmax exp is split
between the ACT engine (exact exp, fp8 output) and the DVE engine
(one-instruction Schraudolph exp: scores * A + B rounded to uint8 IS
the fp8e4 bit pattern).  Denominators come from a ones column packed
into the PV weight tiles; LayerNorm statistics and tail run on Pool.

DoubleRow ISA constraint honored throughout: the stationary operand
must be [p, 2, 128] with the two 128-column slots contiguous (kt tiles
are duplicated via SBUF->SBUF DMA to satisfy this); the moving operand
tolerates arbitrary slot strides including 0 (broadcast).
"""

import os
import sys

os.environ.setdefault("JAX_PLATFORMS", "axon")
sys.path.insert(0, "/opt/trn_rl_repo")

import numpy as np
import ml_dtypes

import concourse.bass as bass
import concourse.tile as tile
from concourse import bacc, mybir
from concourse.bass import ts
from concourse.bass_utils import run_bass_kernel_spmd

N_HEAD = 16
D_MODEL = 1024
D_HEAD = 64
SEQ = 2048
BATCH = 4
EPS = 1e-5
N_CORES = 8

LOCAL = SEQ // 2            # tokens owned per core (1024)
N_PAIR = N_HEAD // 2        # head pairs (8)
JT = SEQ // 128             # j tiles (16)
JG = JT // 2                # j tile pairs (8)

F32 = mybir.dt.float32
FP8 = mybir.dt.float8e4
U8 = mybir.dt.uint8
AF = mybir.ActivationFunctionType
ALU = mybir.AluOpType
DR = mybir.MatmulPerfMode.DoubleRow
E4 = ml_dtypes.float8_e4m3

LOG2E = 1.4426950408889634
WS = 16.0                       # weight quantization scale
EXP_SCALE = 1.0 / 4096.0        # score descale (16*16*8 * 2 dup slots)
EXP_BIAS = -5.0 * float(np.log(2.0))
SCHD_A = 8.0 * LOG2E * EXP_SCALE
SCHD_B = 56.0 + 8.0 * LOG2E * EXP_BIAS   # = 16.0
OUT_DESCALE = 1.0 / 256.0

# drain scheduling: pick the engine whose accumulated queue cost is lowest,
# so consecutive exp tiles alternate ACT/DVE and run concurrently.
ACT_EXP_NS, DVE_EXP_NS = 1038.0, 1330.0
ACT_CPY_NS, DVE_CPY_NS = 1038.0, 1192.0


class _Sched:
    """Cost-balancing chooser between ACT ('a') and DVE ('d')."""

    def __init__(self):
        self.a_ns = 0.0
        self.d_ns = 0.0   # DVE-only tail work is accounted as emitted
        self.late = False

    def pick(self, a_cost=ACT_EXP_NS, d_cost=DVE_EXP_NS):
        if self.late:
            d_cost = d_cost * 1.6
        if self.a_ns + a_cost <= self.d_ns + d_cost:
            self.a_ns += a_cost
            return "a"
        self.d_ns += d_cost
        return "d"

    def dve_extra(self, ns):
        self.d_ns += ns

    def act_extra(self, ns):
        self.a_ns += ns


DEBUG_TAPS = False


def build_program():
    nc = bacc.Bacc()

    hbt = nc.declare_dram_parameter("hbt", [128, 16384], FP8, isOutput=False)
    wq = nc.declare_dram_parameter("wq", [128, 8192], FP8, isOutput=False)
    wk = nc.declare_dram_parameter("wk", [128, 8192], FP8, isOutput=False)
    wv = nc.declare_dram_parameter("wv", [128, 8192], FP8, isOutput=False)
    wo = nc.declare_dram_parameter("wo", [128, 8192], FP8, isOutput=False)
    hbres = nc.declare_dram_parameter("hbres", [LOCAL, D_MODEL], F32,
                                      isOutput=False)
    gamma = nc.declare_dram_parameter("gamma", [D_MODEL], F32, isOutput=False)
    beta = nc.declare_dram_parameter("beta", [D_MODEL], F32, isOutput=False)
    out = nc.declare_dram_parameter("out", [LOCAL, D_MODEL], F32, isOutput=True)
    taps = None
    if DEBUG_TAPS:
        taps = {
            "t_kt2": nc.declare_dram_parameter("t_kt2", [128, 4096], FP8, isOutput=True),
            "t_qt": nc.declare_dram_parameter("t_qt", [128, 1024], FP8, isOutput=True),
            "t_e": nc.declare_dram_parameter("t_e", [128, 1024], FP8, isOutput=True),
            "t_vall": nc.declare_dram_parameter("t_vall", [128, 2048], FP8, isOutput=True),
            "t_at": nc.declare_dram_parameter("t_at", [128, 4096], FP8, isOutput=True),
            "t_rb": nc.declare_dram_parameter("t_rb", [64, 512], F32, isOutput=True),
            "t_acc": nc.declare_dram_parameter("t_acc", [16, 512], F32, isOutput=True),
            "t_rec": nc.declare_dram_parameter("t_rec", [1, 512], F32, isOutput=True),
        }

    with tile.TileContext(nc) as tc:
        with (
            tc.tile_pool(name="consts", bufs=1) as consts,
            tc.tile_pool(name="hbt", bufs=1) as hbt_pool,
            tc.tile_pool(name="wqk", bufs=1) as wqk_pool,
            tc.tile_pool(name="wvo", bufs=1) as wvo_pool,
            tc.tile_pool(name="vall", bufs=1) as v_pool,
            tc.tile_pool(name="kt2", bufs=2) as kt2_pool,
            tc.tile_pool(name="kttmp", bufs=2) as kttmp_pool,
            tc.tile_pool(name="qt", bufs=2) as qt_pool,
            tc.tile_pool(name="e", bufs=12) as e_pool,
            tc.tile_pool(name="attnT", bufs=2) as at_pool,
            tc.tile_pool(name="x", bufs=4) as x_pool,
            tc.tile_pool(name="hbr", bufs=4) as hbr_pool,
            tc.tile_pool(name="small", bufs=4) as sm_pool,
            tc.tile_pool(name="ps_s2", bufs=3, space="PSUM") as ps_s2,
            tc.tile_pool(name="ps_acc", bufs=1, space="PSUM") as ps_acc,
            tc.tile_pool(name="ps_pp", bufs=1, space="PSUM") as ps_pp,
        ):
            _emit(nc, tc, hbt, wq, wk, wv, wo, hbres, gamma, beta, out,
                  consts, hbt_pool, wqk_pool, wvo_pool, v_pool, kt2_pool,
                  kttmp_pool, qt_pool, e_pool, at_pool, x_pool, hbr_pool,
                  sm_pool, ps_s2, ps_acc, ps_pp, taps)

    nc.finalize()
    return nc


def _emit(nc, tc, hbt_d, wq_d, wk_d, wv_d, wo_d, hbres_d, gamma_d, beta_d,
          out_d, consts, hbt_pool, wqk_pool, wvo_pool, v_pool, kt2_pool,
          kttmp_pool, qt_pool, e_pool, at_pool, x_pool, hbr_pool, sm_pool,
          ps_s2, ps_acc, ps_pp, taps=None):
    sched = _Sched()

    # ---- constants ----
    gamma_b = consts.tile([128, D_MODEL], F32, name="gamma_b")
    beta_b = consts.tile([128, D_MODEL], F32, name="beta_b")
    ebias = consts.tile([128, 1], F32, name="ebias")
    eps_t = consts.tile([128, 1], F32, name="eps")
    nc.vector.memset(ebias[:], EXP_BIAS)
    nc.vector.memset(eps_t[:], EPS)
    # ---- weight / activation DMAs (wv + hbt first: V-proj gating) ----
    hbt = hbt_pool.tile([128, 16384], FP8, name="hbt")
    wq_sb = wqk_pool.tile([128, 8192], FP8, name="wq")
    wk_sb = wqk_pool.tile([128, 8192], FP8, name="wk")
    wv_sb = wvo_pool.tile([128, 8192], FP8, name="wv")
    wo_sb = wvo_pool.tile([128, 8192], FP8, name="wo")
    for c in range(4):
        eng = nc.sync if c % 2 == 0 else nc.gpsimd
        eng.dma_start(wv_sb[:, ts(c, 2048)], wv_d[:, ts(c, 2048)])
    for half in range(2):
        for k in range(4):
            off = k * 4096 + half * 2048
            eng = nc.sync if k % 2 == 0 else nc.gpsimd
            eng.dma_start(hbt[:, off:off + 2048], hbt_d[:, off:off + 2048])
    def emit_weight_dmas():
        for c in range(4):
            eng = nc.sync if c % 2 == 0 else nc.gpsimd
            eng.dma_start(wk_sb[:, ts(c, 2048)], wk_d[:, ts(c, 2048)])
            eng.dma_start(wq_sb[:, ts(c, 2048)], wq_d[:, ts(c, 2048)])
            eng.dma_start(wo_sb[:, ts(c, 2048)], wo_d[:, ts(c, 2048)])

    # ---- v_all: [128 j, 16 heads * 8 jpairs * 256]; per 128-block:
    #      cols 0:64 = 16*v, col 64 = 1.0 (denominator), 65:128 = 0 ----
    v_all = v_pool.tile([128, N_HEAD * 2048], FP8, name="v_all")
    va = v_all[:].rearrange("p (b c) -> p b c", c=128)     # [128, 256, 128]
    nc.gpsimd.memset(va[:, :, 64:65], 1.0)
    nc.gpsimd.memset(va[:, :, 65:128], 0.0)

    def hbt_pair(k, tb):
        """hbt [p, 2, 128] weights view for dm-chunk pair k, token block tb."""
        base = k * 4096 + tb * 256
        return hbt[:, base:base + 256].rearrange("p (u t) -> p u t", u=2)

    def k_proj(p):
        """K projection for pair p -> kt_tmp fp8 [128, 2048], then dup DMA."""
        kt_tmp = kttmp_pool.tile([128, 2048], FP8, tag="kttmp", name="kttmp")
        for quarter in range(4):
            pp = ps_pp.tile([128, 512], F32, tag="pp", name="kp")
            for tsub in range(4):
                tb = quarter * 4 + tsub
                for k in range(4):
                    nc.tensor.matmul(
                        pp[:, ts(tsub, 128)],
                        wk_sb[:, p * 1024 + k * 256: p * 1024 + k * 256 + 256]
                        .rearrange("p (u m) -> p u m", u=2),
                        hbt_pair(k, tb),
                        start=(k == 0), stop=(k == 3), perf_mode=DR)
            if sched.pick(612.0, 658.0) == "a":
                nc.scalar.activation(kt_tmp[:, ts(quarter, 512)], pp[:], AF.Copy)
            else:
                nc.vector.tensor_copy(kt_tmp[:, ts(quarter, 512)], pp[:])
        kt2 = kt2_pool.tile([128, 4096], FP8, tag="kt2", name="kt2")
        dst = kt2[:].rearrange("p (b u t) -> p b u t", b=16, u=2)
        src = kt_tmp[:].rearrange("p (b t) -> p b t", b=16)
        nc.sync.dma_start(dst[:, :, 0, :], src)
        nc.sync.dma_start(dst[:, :, 1, :], src)
        return kt2

    def q_proj(p):
        """Q projection for pair p (local 1024 tokens) -> qt fp8 [128, 1024]."""
        qt = qt_pool.tile([128, 1024], FP8, tag="qt", name="qt")
        for half in range(2):
            pp = ps_pp.tile([128, 512], F32, tag="pp", name="qp")
            for tsub in range(4):
                tb = half * 4 + tsub
                for k in range(4):
                    nc.tensor.matmul(
                        pp[:, ts(tsub, 128)],
                        wq_sb[:, p * 1024 + k * 256: p * 1024 + k * 256 + 256]
                        .rearrange("p (u m) -> p u m", u=2),
                        hbt_pair(k, tb),
                        start=(k == 0), stop=(k == 3), perf_mode=DR)
            if sched.pick(612.0, 658.0) == "a":
                nc.scalar.activation(qt[:, ts(half, 512)], pp[:], AF.Copy)
            else:
                nc.vector.tensor_copy(qt[:, ts(half, 512)], pp[:])
        return qt

    # ---- V projection: all 16 heads, out [tok, col] ----
    kq0 = {}
    for tb in range(JT):
        if tb == 2:
            emit_weight_dmas()
        if tb == 8:
            kq0["kt2"] = k_proj(0)
        if tb == 10:
            g_ap, b_ap = gamma_d.ap(), beta_d.ap()
            nc.gpsimd.dma_start(
                out=gamma_b[:],
                in_=bass.AP(tensor=g_ap.tensor, offset=g_ap.offset,
                            ap=[[0, 128], [1, D_MODEL]]))
            nc.gpsimd.dma_start(
                out=beta_b[:],
                in_=bass.AP(tensor=b_ap.tensor, offset=b_ap.offset,
                            ap=[[0, 128], [1, D_MODEL]]))
        if tb == 12:
            kq0["qt"] = q_proj(0)
        pp = ps_s2.tile([128, 1024], F32, tag="s2", name="vp")
        for cg in range(2):
            for k in range(4):
                nc.tensor.matmul(
                    pp[:, ts(cg, 512)],
                    hbt_pair(k, tb),
                    wv_sb[:, 2 * k * 1024: 2 * k * 1024 + 2048]
                    .rearrange("p (u n) -> p u n", u=2)
                    [:, :, cg * 512:(cg + 1) * 512],
                    start=(k == 0), stop=(k == 3), perf_mode=DR)
        # one strided copy: 16 heads' 64-col blocks -> v_all block tb
        dst = v_all[:].rearrange("p (hd b c) -> p hd b c",
                                 hd=N_HEAD, c=128)[:, :, tb, 0:64]
        src = pp[:].rearrange("p (hd c) -> p hd c", c=64)
        if sched.pick() == "a":
            nc.scalar.activation(dst, src, AF.Copy)
        else:
            nc.vector.tensor_copy(dst, src)


    at_tiles = {}   # itile -> attnT tile [128, 4096]

    def attn_scores(p, kt2, qt, itile, h, hook=None):
        """Scores + softmax-exp for (pair p, head h, 512-token itile)."""
        n = 2 * p + h
        e_aps = []
        for g in range(JG):
            s2 = ps_s2.tile([128, 1024], F32, tag="s2", name="s2")
            for u in range(2):
                jc = 2 * g + u
                nc.tensor.matmul(
                    s2[:, ts(u, 512)],
                    kt2[ts(h, 64), jc * 256: jc * 256 + 256]
                    .rearrange("p (u2 t) -> p u2 t", u2=2),
                    qt[ts(h, 64), ts(itile, 512)]
                    .unsqueeze(1).broadcast_to([64, 2, 512]),
                    start=True, stop=True, perf_mode=DR)
            if sched.pick() == "a":
                e_t = e_pool.tile([128, 1024], FP8, tag="e", name="e")
                nc.scalar.activation(e_t[:], s2[:], AF.Exp,
                                     bias=ebias[:], scale=EXP_SCALE)
                e_ap = e_t[:]
            else:
                e_t = e_pool.tile([128, 1024], U8, tag="e", name="e")
                nc.vector.tensor_scalar(e_t[:], s2[:], SCHD_A, SCHD_B,
                                        op0=ALU.mult, op1=ALU.add)
                e_ap = e_t[:].bitcast(FP8)
            e_aps.append(e_ap)
            if hook is not None and g == 3:
                hook()
                hook = None
        if hook is not None:
            hook()
        return (n, itile, h, e_aps)

    norm_q = []

    def flush_norm():
        while norm_q:
            acc0, rb0, p0, itile0, h0 = norm_q.pop(0)
            at_t = at_tiles[itile0]
            dst = at_t[ts(h0, 64), :].rearrange(
                "p (b q t) -> p b q t", b=4, q=8)[:, :, p0, :]
            nc.vector.tensor_tensor(
                dst,
                acc0[0:64, :].rearrange("p (b t) -> p b t", b=4),
                rb0[:].rearrange("p (b t) -> p b t", b=4),
                op=ALU.mult)
            sched.dve_extra(660.0)

    def attn_pv(ctx):
        """PV + normalization for a stream emitted by attn_scores."""
        n, itile, h, e_aps = ctx
        p = n // 2
        flush_norm()
        acc = ps_acc.tile([128, 512], F32, tag="acc", name="acc")
        for g in range(JG):
            nc.tensor.matmul(
                acc[:],
                v_all[:, n * 2048 + g * 256: n * 2048 + g * 256 + 256]
                .rearrange("p (u m) -> p u m", u=2),
                e_aps[g].rearrange("p (u n) -> p u n", u=2),
                start=(g == 0), stop=(g == JG - 1), perf_mode=DR)
        rec = sm_pool.tile([1, 512], F32, tag="rec", name="rec")
        nc.vector.reciprocal(rec[:], acc[64:65, :])
        sched.dve_extra(660.0)
        rb = sm_pool.tile([64, 512], F32, tag="rb", name="rb")
        nc.gpsimd.partition_broadcast(rb[:], rec[:])
        norm_q.append((acc, rb, p, itile, h))

    wo_state = {}

    def wo_mats(itile):
        """Output projection + residual + LN stats for a 512-token block."""
        at_t = at_tiles[itile]
        xs = []
        # mv cols (2*tb4, 2*tb4+1) = (mean, var) per 128-token sub-block
        mv = sm_pool.tile([128, 8], F32, tag="mv", name="mv")
        hbrs = []
        for tb4 in range(4):
            isub = itile * 4 + tb4
            hbr = hbr_pool.tile([128, D_MODEL], F32, tag="hbr", name="hbr")
            nc.gpsimd.dma_start(hbr[:], hbres_d[ts(isub, 128), :])
            hbrs.append(hbr)
        for tb4 in range(4):
            isub = itile * 4 + tb4
            hbr = hbrs[tb4]
            x = x_pool.tile([128, D_MODEL], F32, tag="x", name="x")
            if itile == 1:
                # attention is over: the wide s2 psum tiles are free
                pp = ps_s2.tile([128, 1024], F32, tag="s2", name="op")
                for dm in range(2):
                    for q in range(4):
                        nc.tensor.matmul(
                            pp[:, ts(dm, 512)],
                            at_t[:, tb4 * 1024 + 2 * q * 128:
                                 tb4 * 1024 + 2 * q * 128 + 256]
                            .rearrange("p (u m) -> p u m", u=2),
                            wo_sb[:, 2 * q * 1024: 2 * q * 1024 + 2048]
                            .rearrange("p (u n) -> p u n", u=2)
                            [:, :, dm * 512:(dm + 1) * 512],
                            start=(q == 0), stop=(q == 3), perf_mode=DR)
                nc.vector.scalar_tensor_tensor(
                    x[:], pp[:], OUT_DESCALE, hbr[:],
                    op0=ALU.mult, op1=ALU.add)
                sched.dve_extra(1200.0)
            else:
                for dm in range(2):
                    pp = ps_pp.tile([128, 512], F32, tag="pp", name="op")
                    for q in range(4):
                        nc.tensor.matmul(
                            pp[:],
                            at_t[:, tb4 * 1024 + 2 * q * 128:
                                 tb4 * 1024 + 2 * q * 128 + 256]
                            .rearrange("p (u m) -> p u m", u=2),
                            wo_sb[:, 2 * q * 1024: 2 * q * 1024 + 2048]
                            .rearrange("p (u n) -> p u n", u=2)
                            [:, :, dm * 512:(dm + 1) * 512],
                            start=(q == 0), stop=(q == 3), perf_mode=DR)
                    nc.vector.scalar_tensor_tensor(
                        x[:, ts(dm, 512)], pp[:], OUT_DESCALE,
                        hbr[:, ts(dm, 512)],
                        op0=ALU.mult, op1=ALU.add)
                    sched.dve_extra(660.0)
            xs.append(x)
            # LN stats on DVE (bn_stats/bn_aggr -> mean, var)
            stats = sm_pool.tile([128, 2, 6], F32, tag="bst", name="bst")
            for g2 in range(2):
                nc.vector.bn_stats(stats[:, g2, :], x[:, ts(g2, 512)])
            nc.vector.bn_aggr(mv[:, 2 * tb4:2 * tb4 + 2], stats[:])
            sched.dve_extra(1450.0)
        wo_state[itile] = (xs, mv)

    def wo_tail(itile):
        """rstd + normalize + gamma/beta + output DMA."""
        xs, mv = wo_state.pop(itile)
        var_v = mv[:].rearrange("p (b t) -> p b t", t=2)[:, :, 1:2].squeeze(2)
        rstd = sm_pool.tile([128, 4], F32, tag="rstd", name="rstd")
        rvar = sm_pool.tile([128, 4], F32, tag="rvar", name="rvar")
        nc.vector.tensor_scalar(rvar[:], var_v, 1.0, EPS,
                                op0=ALU.mult, op1=ALU.add)
        nc.vector.reciprocal(rstd[:], rvar[:])
        nc.scalar.activation(rstd[:], rstd[:], AF.Sqrt)
        sched.act_extra(400.0)
        for tb4 in range(4):
            isub = itile * 4 + tb4
            x = xs[tb4]
            # fused LN tail on DVE: t = (x - mu) * gamma ; y = t * rstd + beta
            nc.vector.scalar_tensor_tensor(
                x[:], x[:], mv[:, 2 * tb4:2 * tb4 + 1], gamma_b[:],
                op0=ALU.subtract, op1=ALU.mult)
            nc.vector.scalar_tensor_tensor(
                x[:], x[:], rstd[:, tb4:tb4 + 1], beta_b[:],
                op0=ALU.mult, op1=ALU.add)
            sched.dve_extra(1200.0)
            eng = nc.sync if tb4 % 2 == 0 else nc.gpsimd
            eng.dma_start(out_d[ts(isub, 128), :], x[:])

    # ---- main pair loop with cross-pair pipelining ----
    kt2 = kq0["kt2"]
    qt = kq0["qt"]
    at_tiles[0] = at_pool.tile([128, 4096], FP8, tag="at0", name="at0")
    at_tiles[1] = at_pool.tile([128, 4096], FP8, tag="at1", name="at1")

    next_ref = {}

    if taps is not None:
        nc.sync.dma_start(taps["t_kt2"][:, :], kt2[:])
        nc.sync.dma_start(taps["t_qt"][:, :], qt[:])
        nc.sync.dma_start(taps["t_vall"][:, :], v_all[:, 0:2048])

    pending = []
    for p in range(N_PAIR):
        if p == N_PAIR - 1:
            sched.late = True
        hooks = {}
        if p + 1 < N_PAIR:
            pn = p + 1
            hooks[(0, 0)] = lambda pn=pn: next_ref.__setitem__(
                "kt2", k_proj(pn))
            hooks[(1, 0)] = lambda pn=pn: next_ref.__setitem__(
                "qt", q_proj(pn))
        else:
            def _wo0_hook():
                while pending:
                    attn_pv(pending.pop(0))
                flush_norm()
                wo_mats(0)
            hooks[(1, 0)] = _wo0_hook
        for itile in range(2):
            for h in range(2):
                ctx = attn_scores(p, kt2, qt, itile, h,
                                  hook=hooks.pop((itile, h), None))
                pending.append(ctx)
                if len(pending) > 1:
                    attn_pv(pending.pop(0))
        if p + 1 < N_PAIR:
            kt2 = next_ref.pop("kt2")
            qt = next_ref.pop("qt")
    while pending:
        attn_pv(pending.pop(0))
    flush_norm()
    wo_tail(0)
    wo_mats(1)
    wo_tail(1)
    if taps is not None:
        nc.sync.dma_start(taps["t_at"][:, :], at_tiles[0][:])


_program_cache = {}


def _get_program():
    if "nc" not in _program_cache:
        _program_cache["nc"] = build_program()
    return _program_cache["nc"]


def _q8(x):
    return np.ascontiguousarray(np.asarray(x, np.float32).astype(E4))


def _shard_inputs(h, Wq, Wkv, Wo, gamma, beta):
    """Build the 8 per-core input maps (host-side numpy only)."""
    h = np.asarray(h, np.float32)
    Wq = np.asarray(Wq, np.float32) * WS
    Wkv = np.asarray(Wkv, np.float32) * WS
    Wo = np.asarray(Wo, np.float32) * WS
    gamma = np.asarray(gamma, np.float32)
    beta = np.asarray(beta, np.float32)

    Wk = Wkv[:, :N_HEAD * D_HEAD]
    Wv = Wkv[:, N_HEAD * D_HEAD:]

    def pack_qk(W):
        # w8[p, cb*1024 + k*256 + u*128 + col] = W[k*256+u*128+p, cb*128+col]
        B = _q8(W).reshape(4, 2, 128, 8, 128)        # [k, u, p, cb, col]
        return np.ascontiguousarray(
            B.transpose(2, 3, 0, 1, 4).reshape(128, 8192))

    def pack_rowmajor(W):
        # w8[p, c*1024 + col] = W[128c+p, col]
        C = _q8(W).reshape(8, 128, 1024)             # [c, p, col]
        return np.ascontiguousarray(C.transpose(1, 0, 2).reshape(128, 8192))

    wq8 = pack_qk(Wq)
    wk8 = pack_qk(Wk)
    wv8 = pack_rowmajor(Wv)
    wo8 = pack_rowmajor(Wo)

    in_maps = []
    for core in range(N_CORES):
        b, r = divmod(core, 2)
        hb_full = h[:, b, :]
        if r == 0:
            hb_perm = hb_full
        else:
            hb_perm = np.concatenate([hb_full[LOCAL:], hb_full[:LOCAL]], 0)
        # hbt8[p, k*4096 + tb*256 + u*128 + t] = hb_perm[tb*128+t, 256k+128u+p]
        A = _q8(hb_perm).reshape(16, 128, 4, 2, 128)  # [tb, t, k, u, p]
        hbt8 = np.ascontiguousarray(
            A.transpose(4, 2, 0, 3, 1).reshape(128, 16384))
        in_maps.append({
            "hbt": hbt8,
            "wq": wq8, "wk": wk8, "wv": wv8, "wo": wo8,
            "hbres": np.ascontiguousarray(hb_perm[:LOCAL]),
            "gamma": gamma, "beta": beta,
        })
    return in_maps


def kernel(h, Wq, Wkv, Wo, gamma, beta, _trace=False):
    nc = _get_program()
    in_maps = _shard_inputs(h, Wq, Wkv, Wo, gamma, beta)
    res = run_bass_kernel_spmd(nc, in_maps, list(range(N_CORES)), trace=_trace)
    if _trace:
        kernel.last_results = res

    out = np.empty((SEQ, BATCH, D_MODEL), np.float32)
    for core in range(N_CORES):
        b, r = divmod(core, 2)
        out[r * LOCAL:(r + 1) * LOCAL, b, :] = res.results[core]["out"]
    return out
